# revision 16
# baseline (speedup 1.0000x reference)
"""Trainium2 Bass kernel for nn_ARMAPosteriorModel (blocked-matmul design).

The reference's windowed ARMA computation is a first-order linear recurrence
over time:

    ap[t] = sigmoid(a_raw)[t-1]      (ap[0] = 0)
    z[s,t] = mean[t] + s[t]*noise[s,t]
    param[s,t] = ap[t]*param[s,t-1] + z[s,t]
    lp[s,t] = -log(s[t]) - 0.5*log(2*pi) - 0.5*noise[s,t]^2

Instead of a DVE scan (2.16 ns/col, 17.7us/core), the recurrence is computed
as block-triangular matmuls on the tensor engine:

    param[bs+k] = sum_j L[k,j] z[bs+j]  +  (strip from last 16 rows of the
                                            previous block)

where L[k,j] = prod(ap[bs+j+1 .. bs+k]) is host-precomputed per (block, dim).
Contributions older than ~16+128 steps decay below 2e-4 (verified exactly on
the host against a scanned upper bound); where even the 16-deep strip is not
needed it is omitted.

Device layout: partition = t within a 128-block, free = (block-pair, d, p, s)
so the per-(t,d,p) parameters broadcast over s via stride-0 access patterns.
Per core: 4 super-tiles of (128, 2048) covering 2 t-blocks each; 32 local
samples (data-parallel over S across 8 cores).
"""

import sys

if "/opt/trn_rl_repo" not in sys.path:
    sys.path.insert(0, "/opt/trn_rl_repo")

import numpy as np

N_CORES = 8
S = 256
T = 1024
D = 4
P = 8
S_LOCAL = S // N_CORES       # 32 samples per core
B = 128                      # t-block size (= matmul contraction/out size)
NB = T // B                  # 8 blocks
NSUP = NB // 2               # 4 super-tiles of 2 blocks
FB = D * P * S_LOCAL         # free width per block = 1024
Q = 64                       # strip contraction depth (base-64 rhs)
QR = 16                      # nonzero strip rows actually shipped
LOG2PI = float(np.log(2.0 * np.pi))

_NC_CACHE = {}


def _build_bass(strip_mask):
    """strip_mask: tuple of NB*D bools, index blk*D+d (blk>=1)."""
    import concourse.tile as tile
    from concourse import bacc, mybir

    nc = bacc.Bacc(
        "TRN2", target_bir_lowering=False, debug=False, num_devices=N_CORES
    )
    f16 = mybir.dt.float16
    f32 = mybir.dt.float32
    mult = mybir.AluOpType.mult
    add = mybir.AluOpType.add
    subtract = mybir.AluOpType.subtract
    Sq = mybir.ActivationFunctionType.Square

    W2 = 2 * FB              # super-tile width 2048

    noise_in = nc.dram_tensor("noise", [NSUP, 128, W2], f16, kind="ExternalInput")
    # per-block (128, 32) minitiles for s, mean, nnl; packed [128, 3*NB*32]
    mini_in = nc.dram_tensor("mini", [128, 3 * NB * 32], f16, kind="ExternalInput")
    # diag weights, lhsT layout (u, t): per (blk, d) a (128,128) slab
    wdiag_in = nc.dram_tensor("wdiag", [NSUP, 128, 2 * D * B], f16,
                              kind="ExternalInput")
    nstrip = int(sum(strip_mask))
    # only the last QR rows of the 64-deep strip window carry weight
    wstrip_in = nc.dram_tensor("wstrip", [QR, max(nstrip, 1) * B], f16,
                               kind="ExternalInput")
    param_out = nc.dram_tensor("param", [NSUP, 128, W2], f16, kind="ExternalOutput")
    lp_out = nc.dram_tensor("lp", [NSUP, 128, W2], f16, kind="ExternalOutput")

    def bcast_mini(ap128x32):
        # (128, 32) minitile (col = d*8+p) -> (128, 4, 32, 8): s broadcast on
        # the middle dim so inner runs stay contiguous (full DVE rate)
        return ap128x32.rearrange("p (d q) -> p d q", d=4).unsqueeze(2) \
            .broadcast_to((128, 4, 32, 8))

    def dsp(ap_seg):
        # (128, 1024) block segment -> (128, 4, 32, 8) real strides
        return ap_seg.rearrange("p (d s q) -> p d s q", d=4, s=32)

    with tile.TileContext(nc) as tc:
        with (
            tc.tile_pool(name="const", bufs=1) as cpool,
            tc.tile_pool(name="nin", bufs=3) as npool,
            tc.tile_pool(name="wgt", bufs=2) as wpool,
            tc.tile_pool(name="zt", bufs=3) as zpool,
            tc.tile_pool(name="sqp", bufs=3) as qpool,
            tc.tile_pool(name="outp", bufs=3) as opool,
            tc.tile_pool(name="ps", bufs=2, space="PSUM") as pspool,
        ):
            MINI = cpool.tile([128, 3 * NB * 32], f16, tag="mini", name="mini_t")
            nc.scalar.dma_start(MINI[:], mini_in[:])

            def mini_ap(c, blk):
                # (128, 32) slice of constant c for block blk
                return MINI[:, c * NB * 32 + blk * 32:c * NB * 32 + (blk + 1) * 32]

            strips = {}
            si = 0
            for blk in range(1, NB):
                for d in range(D):
                    if strip_mask[blk * D + d]:
                        strips[(blk, d)] = si
                        si += 1
            # PE warm-up: dummy matmuls on a memset tile while DMAs load,
            # so HAM reaches 2.4 GHz before the real matmuls issue.
            SCR = cpool.tile([128, 512], f16, tag="scr", name="scr_t")
            nc.gpsimd.memset(SCR[:], 0.0)

            # scalar-ring issue order is FIFO: load super-0's weights before
            # the bulky strip/later weights so the first matmuls are not
            # stuck behind low-priority transfers.
            # Each noise tile is split across both HWDGE rings so the
            # half feeding the next matmul lands in half the time; issue
            # order is need-ordered (super 0 first).
            nts, wds = [], []
            for i in range(NSUP):
                nt = npool.tile([128, W2], f16, tag="noise", name=f"nt{i}")
                nts.append(nt)
                wd = wpool.tile([128, 2 * D * B], f16, tag="wd", name=f"wd{i}")
                wds.append(wd)
            WS = None
            if nstrip:
                # strip lhsT must share the rhs base partition (64); only the
                # last QR rows are nonzero — ship those, zero the rest.
                WS = cpool.tile([128, nstrip * B], f16, tag="ws", name="ws_t")
                nc.gpsimd.memset(WS[64:128 - QR, :], 0.0)
            nc.sync.dma_start(nts[0][:, 0:FB], noise_in[0, :, 0:FB])
            nc.scalar.dma_start(wds[0][:], wdiag_in[0])
            nc.scalar.dma_start(nts[0][:, FB:W2], noise_in[0, :, FB:W2])
            if nstrip:
                nc.scalar.dma_start(WS[128 - QR:128, :], wstrip_in[:])
            for i in range(1, NSUP):
                nc.sync.dma_start(nts[i][:, 0:FB], noise_in[i, :, 0:FB])
                nc.scalar.dma_start(wds[i][:], wdiag_in[i])
                nc.scalar.dma_start(nts[i][:, FB:W2], noise_in[i, :, FB:W2])

            wps = pspool.tile([128, W2], mybir.dt.float32, tag="ps",
                              name="warm_ps")
            for w in range(12):
                nc.tensor.matmul(
                    wps[:, 0:256], SCR[:, 0:128], SCR[:, 0:256],
                    start=True, stop=True, skip_group_check=True)

            zts = []
            psums, pts = [], []

            def emit_copy(j):
                pt = opool.tile([128, W2], f16, tag="param", name=f"pt{j}")
                nc.scalar.mul(pt[:], psums[j][:], 1.0)
                nc.sync.dma_start(param_out[j], pt[:])

            for i in range(NSUP):
                nt = nts[i]
                wd = wds[i]
                # --- z = mean + s*noise (feeds PE; first on DVE) ---
                zt = zpool.tile([128, W2], f16, tag="z", name=f"zt{i}")
                zts.append(zt)
                for b2 in range(2):
                    blk = 2 * i + b2
                    zs = dsp(zt[:, b2 * FB:(b2 + 1) * FB])
                    ns = dsp(nt[:, b2 * FB:(b2 + 1) * FB])
                    nc.vector.tensor_tensor(
                        zs, ns, bcast_mini(mini_ap(0, blk)), mult)
                    nc.vector.tensor_tensor(
                        zs, zs, bcast_mini(mini_ap(1, blk)), add)

                # --- lp path (sq early on ACT; the psum copy of the
                # previous super is emitted after it so ACT never blocks
                # the next square behind a PE-gated copy) ---
                sq = qpool.tile([128, W2], f16, tag="sq", name=f"sq{i}")
                nc.scalar.activation(sq[:], nt[:], Sq, scale=0.7071067811865476)
                if i > 0:
                    emit_copy(i - 1)
                lt = opool.tile([128, W2], f16, tag="lp", name=f"lt{i}")
                for b2 in range(2):
                    blk = 2 * i + b2
                    lslice = dsp(lt[:, b2 * FB:(b2 + 1) * FB])
                    sqs = dsp(sq[:, b2 * FB:(b2 + 1) * FB])
                    nc.vector.tensor_tensor(
                        lslice, bcast_mini(mini_ap(2, blk)), sqs, subtract)
                (nc.scalar if i % 2 == 0 else nc.sync).dma_start(
                    lp_out[i], lt[:])

                # --- param blocks via PE ---
                psum = pspool.tile([128, W2], f32, tag="ps", name=f"ps{i}")
                for b2 in range(2):
                    blk = 2 * i + b2
                    # z tile and column offset holding the previous block
                    if b2 == 1:
                        zpt, zpo = zt, 0
                    elif i > 0:
                        zpt, zpo = zts[i - 1], FB
                    else:
                        zpt, zpo = None, 0
                    for d in range(D):
                        out = psum[:, b2 * FB + d * 256:b2 * FB + (d + 1) * 256]
                        rhs = zt[:, b2 * FB + d * 256:b2 * FB + (d + 1) * 256]
                        sidx = strips.get((blk, d))
                        if sidx is not None and zpt is not None:
                            nc.tensor.matmul(
                                out,
                                WS[64:128, sidx * B:(sidx + 1) * B],
                                zpt[B - Q:B,
                                    zpo + d * 256:zpo + (d + 1) * 256],
                                start=True, stop=False,
                                skip_group_check=True,
                            )
                            nc.tensor.matmul(
                                out,
                                wd[:, (b2 * D + d) * B:(b2 * D + d + 1) * B],
                                rhs,
                                start=False, stop=True,
                                skip_group_check=True,
                            )
                        else:
                            nc.tensor.matmul(
                                out,
                                wd[:, (b2 * D + d) * B:(b2 * D + d + 1) * B],
                                rhs,
                                start=True, stop=True,
                            )
                psums.append(psum)
            emit_copy(NSUP - 1)
    nc.finalize()
    return nc


def _get_nc(strip_mask):
    key = tuple(strip_mask)
    if key not in _NC_CACHE:
        _NC_CACHE[key] = _build_bass(key)
    return _NC_CACHE[key]


def _host_prep(m, s_raw, a_raw, noise, dim_idx):
    """Returns (mini, wdiag, wstrip, strip_mask).

    mini: (3, 128, NB*32) f16 — [s, mean, nnl] minitiles per block
    wdiag: (NSUP, 128, 2*D*B) f16 — lhsT[u, t] = L[t, u] per (blk, d)
    wstrip: (nstrip, Q, B) f16
    """
    mm = np.asarray(m)[:, dim_idx].astype(np.float64)          # (T,D,P)
    sr = np.asarray(s_raw)[:, dim_idx].astype(np.float64)
    ar = np.asarray(a_raw)[:, dim_idx, 0].astype(np.float64)   # (T-1,D)

    s = np.logaddexp(0.0, sr)
    ap = np.zeros((T, D))
    ap[1:] = 1.0 / (1.0 + np.exp(-ar))
    mean = (1.0 - ap)[:, :, None] * mm
    nnl = -np.log(s) - 0.5 * LOG2PI

    # minitiles: row = t within block (partition), col = blk*32 + d*P + p
    def to_mini(x):  # (T,D,P) -> (128, NB*32)
        return np.ascontiguousarray(
            x.reshape(NB, B, D * P).transpose(1, 0, 2).reshape(128, NB * 32)
        )

    mini = np.concatenate(
        [to_mini(s), to_mini(mean), to_mini(nnl)], axis=1).astype(np.float16)

    # exact residual bound -> strip mask
    nmax = np.abs(np.asarray(noise)).max(axis=(0, 3))          # (T,D)
    zb = np.abs(mean).max(axis=2) + s.max(axis=2) * nmax       # (T,D)
    Wb = np.zeros((T, D))
    acc = np.zeros(D)
    for t in range(T):
        acc = ap[t] * acc + zb[t]
        Wb[t] = acc

    wdiag = np.zeros((NSUP, 128, 2 * D * B), np.float16)
    strip_mask = [False] * (NB * D)
    wstrips = []
    tril = np.tril(np.ones((B, B), bool))
    for blk in range(NB):
        bs = blk * B
        i, b2 = blk // 2, blk % 2
        for d in range(D):
            apb = ap[bs:bs + B, d]
            Pk = np.ones(B)
            Pk[1:] = np.cumprod(apb[1:])
            with np.errstate(divide="ignore", invalid="ignore"):
                Lb = Pk[:, None] / Pk[None, :]
            Lb = np.nan_to_num(np.where(tril, Lb, 0.0), posinf=0.0, neginf=0.0)
            wdiag[i, :, (b2 * D + d) * B:(b2 * D + d + 1) * B] = \
                Lb.T.astype(np.float16)
            if blk == 0:
                continue
            ps = bs - B
            app = ap[ps:ps + B, d]
            Pp = np.ones(B)
            Pp[1:] = np.cumprod(app[1:])
            with np.errstate(divide="ignore", invalid="ignore"):
                tailp = np.nan_to_num(Pp[B - 1] / Pp, posinf=0.0, neginf=0.0)
            colk = ap[bs, d] * Pk
            Ls = np.outer(tailp, colk)                         # [j, k] lhsT
            longmax = (ap[ps, d] * Pp[B - 1] * colk).max() * Wb[max(ps - 1, 0), d]
            resid_with = (Ls[:B - QR].T @ zb[ps:ps + B - QR, d]).max() + longmax
            resid_no = (Ls.T @ zb[ps:ps + B, d]).max() + longmax
            if resid_no > 2e-4:
                assert resid_with < 2e-3, (
                    f"strip depth {QR} insufficient: {resid_with}")
                strip_mask[blk * D + d] = True
                wstrips.append(Ls[B - QR:B].astype(np.float16))
    nstrip = len(wstrips)
    wstrip = (np.concatenate(wstrips, axis=1) if nstrip
              else np.zeros((QR, B), np.float16))
    return mini, wdiag, np.ascontiguousarray(wstrip), tuple(strip_mask)


def _noise_dev_layout(noise_core):
    """(S_LOCAL, T, D, P) f32 -> (NSUP, 128, 2048) f16, free = (d, s, p)."""
    x = noise_core.transpose(1, 2, 0, 3)           # (T, D, S_LOCAL, P)
    x = x.reshape(NB, B, FB)                       # (blk, tt, dsp)
    x = x.reshape(NSUP, 2, B, FB).transpose(0, 2, 1, 3)  # (i, tt, b2, dsp)
    return np.ascontiguousarray(x.reshape(NSUP, 128, W2_const)).astype(np.float16)


W2_const = 2 * FB


def _undo_layout(dev):
    """(NSUP, 128, 2048) -> (S_LOCAL, T, D, P) float32; free = (d, s, p)."""
    x = dev.reshape(NSUP, B, 2, D, S_LOCAL, P).transpose(0, 2, 1, 3, 4, 5)
    x = x.reshape(T, D, S_LOCAL, P).transpose(2, 0, 1, 3)
    return x.astype(np.float32)


def kernel(
    y=None,
    age=None,
    m=None,
    s_raw=None,
    a_raw=None,
    noise=None,
    cond_sample=None,
    dim_idx=None,
    compute_log_prob=1,
    _trace=False,
    **_unused,
):
    from concourse.bass_utils import run_bass_kernel_spmd

    noise = np.asarray(noise, dtype=np.float32)
    dim_idx = np.asarray(dim_idx)
    mini, wdiag, wstrip, strip_mask = _host_prep(m, s_raw, a_raw, noise, dim_idx)
    nc = _get_nc(strip_mask)

    in_maps = []
    for c in range(N_CORES):
        shard = noise[S_LOCAL * c:S_LOCAL * (c + 1)]
        in_maps.append({
            "noise": _noise_dev_layout(shard),
            "mini": mini,
            "wdiag": wdiag,
            "wstrip": wstrip,
        })

    kw = {}
    if _trace:
        kw = dict(trace=True, trace_cores=list(range(N_CORES)))
    res = run_bass_kernel_spmd(nc, in_maps, core_ids=list(range(N_CORES)), **kw)

    param = np.empty((S, T, D, P), np.float32)
    lp = np.empty((S, T, D, P), np.float32)
    for c in range(N_CORES):
        out = res.results[c]
        sl = slice(S_LOCAL * c, S_LOCAL * (c + 1))
        param[sl] = _undo_layout(out["param"])
        lp[sl] = _undo_layout(out["lp"])
    kernel.last_results = res
    if compute_log_prob:
        return (param, lp)
    return param


# revision 17
# speedup vs baseline: 1.0781x; 1.0781x over previous
"""Trainium2 Bass kernel for nn_ARMAPosteriorModel (blocked-matmul design).

The reference's windowed ARMA computation is a first-order linear recurrence
over time:

    ap[t] = sigmoid(a_raw)[t-1]      (ap[0] = 0)
    z[s,t] = mean[t] + s[t]*noise[s,t]
    param[s,t] = ap[t]*param[s,t-1] + z[s,t]
    lp[s,t] = -log(s[t]) - 0.5*log(2*pi) - 0.5*noise[s,t]^2

Instead of a DVE scan (2.16 ns/col, 17.7us/core), the recurrence is computed
as block-triangular matmuls on the tensor engine:

    param[bs+k] = sum_j L[k,j] z[bs+j]  +  (strip from last 16 rows of the
                                            previous block)

where L[k,j] = prod(ap[bs+j+1 .. bs+k]) is host-precomputed per (block, dim).
Contributions older than ~16+128 steps decay below 2e-4 (verified exactly on
the host against a scanned upper bound); where even the 16-deep strip is not
needed it is omitted.

Device layout: partition = t within a 128-block, free = (block-pair, d, p, s)
so the per-(t,d,p) parameters broadcast over s via stride-0 access patterns.
Per core: 4 super-tiles of (128, 2048) covering 2 t-blocks each; 32 local
samples (data-parallel over S across 8 cores).
"""

import sys

if "/opt/trn_rl_repo" not in sys.path:
    sys.path.insert(0, "/opt/trn_rl_repo")

import numpy as np

N_CORES = 8
S = 256
T = 1024
D = 4
P = 8
S_LOCAL = S // N_CORES       # 32 samples per core
B = 128                      # t-block size (= matmul contraction/out size)
NB = T // B                  # 8 blocks
NSUP = NB // 2               # 4 super-tiles of 2 blocks
FB = D * P * S_LOCAL         # free width per block = 1024
Q = 64                       # strip contraction depth (base-64 rhs)
QR = 16                      # nonzero strip rows actually shipped
LOG2PI = float(np.log(2.0 * np.pi))

_NC_CACHE = {}


def _build_bass(strip_mask):
    """strip_mask: tuple of NB*D bools, index blk*D+d (blk>=1)."""
    import concourse.tile as tile
    from concourse import bacc, mybir

    nc = bacc.Bacc(
        "TRN2", target_bir_lowering=False, debug=False, num_devices=N_CORES
    )
    f16 = mybir.dt.float16
    f32 = mybir.dt.float32
    mult = mybir.AluOpType.mult
    add = mybir.AluOpType.add
    subtract = mybir.AluOpType.subtract
    Sq = mybir.ActivationFunctionType.Square

    W2 = 2 * FB              # super-tile width 2048

    noise_in = nc.dram_tensor("noise", [NSUP, 128, W2], f16, kind="ExternalInput")
    # per-block (128, 32) minitiles for s, mean, nnl; packed [128, 3*NB*32]
    mini_in = nc.dram_tensor("mini", [128, 3 * NB * 32], f16, kind="ExternalInput")
    # diag weights, lhsT layout (u, t): per (blk, d) a (128,128) slab
    wdiag_in = nc.dram_tensor("wdiag", [NSUP, 128, 2 * D * B], f16,
                              kind="ExternalInput")
    nstrip = int(sum(strip_mask))
    # only the last QR rows of the 64-deep strip window carry weight
    wstrip_in = nc.dram_tensor("wstrip", [QR, max(nstrip, 1) * B], f16,
                               kind="ExternalInput")
    param_out = nc.dram_tensor("param", [NSUP, 128, W2], f16, kind="ExternalOutput")
    lp_out = nc.dram_tensor("lp", [NSUP, 128, W2], f16, kind="ExternalOutput")

    def bcast_mini(ap128x32):
        # (128, 32) minitile (col = d*8+p) -> (128, 4, 32, 8): s broadcast on
        # the middle dim so inner runs stay contiguous (full DVE rate)
        return ap128x32.rearrange("p (d q) -> p d q", d=4).unsqueeze(2) \
            .broadcast_to((128, 4, 32, 8))

    def dsp(ap_seg):
        # (128, 1024) block segment -> (128, 4, 32, 8) real strides
        return ap_seg.rearrange("p (d s q) -> p d s q", d=4, s=32)

    with tile.TileContext(nc) as tc:
        with (
            tc.tile_pool(name="const", bufs=1) as cpool,
            tc.tile_pool(name="nin", bufs=3) as npool,
            tc.tile_pool(name="wgt", bufs=2) as wpool,
            tc.tile_pool(name="zt", bufs=3) as zpool,
            tc.tile_pool(name="sqp", bufs=3) as qpool,
            tc.tile_pool(name="outp", bufs=3) as opool,
            tc.tile_pool(name="ps", bufs=2, space="PSUM") as pspool,
        ):
            MINI = cpool.tile([128, 3 * NB * 32], f16, tag="mini", name="mini_t")
            nc.scalar.dma_start(MINI[:], mini_in[:])

            def mini_ap(c, blk):
                # (128, 32) slice of constant c for block blk
                return MINI[:, c * NB * 32 + blk * 32:c * NB * 32 + (blk + 1) * 32]

            strips = {}
            si = 0
            for blk in range(1, NB):
                for d in range(D):
                    if strip_mask[blk * D + d]:
                        strips[(blk, d)] = si
                        si += 1
            # PE warm-up: dummy matmuls on a memset tile while DMAs load,
            # so HAM reaches 2.4 GHz before the real matmuls issue.
            SCR = cpool.tile([128, 512], f16, tag="scr", name="scr_t")
            nc.gpsimd.memset(SCR[:], 0.0)

            # scalar-ring issue order is FIFO: load super-0's weights before
            # the bulky strip/later weights so the first matmuls are not
            # stuck behind low-priority transfers.
            # Each noise tile is split across both HWDGE rings so the
            # half feeding the next matmul lands in half the time; issue
            # order is need-ordered (super 0 first).
            nts, wds = [], []
            for i in range(NSUP):
                nt = npool.tile([128, W2], f16, tag="noise", name=f"nt{i}")
                nts.append(nt)
                wd = wpool.tile([128, 2 * D * B], f16, tag="wd", name=f"wd{i}")
                wds.append(wd)
            WS = None
            if nstrip:
                # strip lhsT must share the rhs base partition (64); only the
                # last QR rows are nonzero — ship those, zero the rest.
                WS = cpool.tile([128, nstrip * B], f16, tag="ws", name="ws_t")
                nc.gpsimd.memset(WS[64:128 - QR, :], 0.0)
            nc.sync.dma_start(nts[0][:], noise_in[0])
            nc.scalar.dma_start(wds[0][:], wdiag_in[0])
            if nstrip:
                nc.scalar.dma_start(WS[128 - QR:128, :], wstrip_in[:])
            for i in range(1, NSUP):
                nc.sync.dma_start(nts[i][:], noise_in[i])
                nc.scalar.dma_start(wds[i][:], wdiag_in[i])

            wps = pspool.tile([128, W2], mybir.dt.float32, tag="ps",
                              name="warm_ps")
            for w in range(12):
                nc.tensor.matmul(
                    wps[:, 0:256], SCR[:, 0:128], SCR[:, 0:256],
                    start=True, stop=True, skip_group_check=True)

            zts = []
            psums, pts = [], []

            def emit_copy(j):
                pt = opool.tile([128, W2], f16, tag="param", name=f"pt{j}")
                nc.scalar.mul(pt[:], psums[j][:], 1.0)
                nc.sync.dma_start(param_out[j], pt[:])

            for i in range(NSUP):
                nt = nts[i]
                wd = wds[i]
                # --- z = mean + s*noise (feeds PE; first on DVE) ---
                zt = zpool.tile([128, W2], f16, tag="z", name=f"zt{i}")
                zts.append(zt)
                for b2 in range(2):
                    blk = 2 * i + b2
                    zs = dsp(zt[:, b2 * FB:(b2 + 1) * FB])
                    ns = dsp(nt[:, b2 * FB:(b2 + 1) * FB])
                    nc.vector.tensor_tensor(
                        zs, ns, bcast_mini(mini_ap(0, blk)), mult)
                    nc.vector.tensor_tensor(
                        zs, zs, bcast_mini(mini_ap(1, blk)), add)

                # --- lp path (sq early on ACT; the psum copy of the
                # previous super is emitted after it so ACT never blocks
                # the next square behind a PE-gated copy) ---
                sq = qpool.tile([128, W2], f16, tag="sq", name=f"sq{i}")
                nc.scalar.activation(sq[:], nt[:], Sq, scale=0.7071067811865476)
                if i > 0:
                    emit_copy(i - 1)
                lt = opool.tile([128, W2], f16, tag="lp", name=f"lt{i}")
                for b2 in range(2):
                    blk = 2 * i + b2
                    lslice = dsp(lt[:, b2 * FB:(b2 + 1) * FB])
                    sqs = dsp(sq[:, b2 * FB:(b2 + 1) * FB])
                    nc.vector.tensor_tensor(
                        lslice, bcast_mini(mini_ap(2, blk)), sqs, subtract)
                nc.scalar.dma_start(lp_out[i], lt[:])

                # --- param blocks via PE ---
                psum = pspool.tile([128, W2], f32, tag="ps", name=f"ps{i}")
                for b2 in range(2):
                    blk = 2 * i + b2
                    # z tile and column offset holding the previous block
                    if b2 == 1:
                        zpt, zpo = zt, 0
                    elif i > 0:
                        zpt, zpo = zts[i - 1], FB
                    else:
                        zpt, zpo = None, 0
                    for d in range(D):
                        out = psum[:, b2 * FB + d * 256:b2 * FB + (d + 1) * 256]
                        rhs = zt[:, b2 * FB + d * 256:b2 * FB + (d + 1) * 256]
                        sidx = strips.get((blk, d))
                        if sidx is not None and zpt is not None:
                            nc.tensor.matmul(
                                out,
                                WS[64:128, sidx * B:(sidx + 1) * B],
                                zpt[B - Q:B,
                                    zpo + d * 256:zpo + (d + 1) * 256],
                                start=True, stop=False,
                                skip_group_check=True,
                            )
                            nc.tensor.matmul(
                                out,
                                wd[:, (b2 * D + d) * B:(b2 * D + d + 1) * B],
                                rhs,
                                start=False, stop=True,
                                skip_group_check=True,
                            )
                        else:
                            nc.tensor.matmul(
                                out,
                                wd[:, (b2 * D + d) * B:(b2 * D + d + 1) * B],
                                rhs,
                                start=True, stop=True,
                            )
                psums.append(psum)
            emit_copy(NSUP - 1)
    nc.finalize()
    return nc


def _get_nc(strip_mask):
    key = tuple(strip_mask)
    if key not in _NC_CACHE:
        _NC_CACHE[key] = _build_bass(key)
    return _NC_CACHE[key]


def _host_prep(m, s_raw, a_raw, noise, dim_idx):
    """Returns (mini, wdiag, wstrip, strip_mask).

    mini: (3, 128, NB*32) f16 — [s, mean, nnl] minitiles per block
    wdiag: (NSUP, 128, 2*D*B) f16 — lhsT[u, t] = L[t, u] per (blk, d)
    wstrip: (nstrip, Q, B) f16
    """
    mm = np.asarray(m)[:, dim_idx].astype(np.float64)          # (T,D,P)
    sr = np.asarray(s_raw)[:, dim_idx].astype(np.float64)
    ar = np.asarray(a_raw)[:, dim_idx, 0].astype(np.float64)   # (T-1,D)

    s = np.logaddexp(0.0, sr)
    ap = np.zeros((T, D))
    ap[1:] = 1.0 / (1.0 + np.exp(-ar))
    mean = (1.0 - ap)[:, :, None] * mm
    nnl = -np.log(s) - 0.5 * LOG2PI

    # minitiles: row = t within block (partition), col = blk*32 + d*P + p
    def to_mini(x):  # (T,D,P) -> (128, NB*32)
        return np.ascontiguousarray(
            x.reshape(NB, B, D * P).transpose(1, 0, 2).reshape(128, NB * 32)
        )

    mini = np.concatenate(
        [to_mini(s), to_mini(mean), to_mini(nnl)], axis=1).astype(np.float16)

    # exact residual bound -> strip mask
    nmax = np.abs(np.asarray(noise)).max(axis=(0, 3))          # (T,D)
    zb = np.abs(mean).max(axis=2) + s.max(axis=2) * nmax       # (T,D)
    Wb = np.zeros((T, D))
    acc = np.zeros(D)
    for t in range(T):
        acc = ap[t] * acc + zb[t]
        Wb[t] = acc

    wdiag = np.zeros((NSUP, 128, 2 * D * B), np.float16)
    strip_mask = [False] * (NB * D)
    wstrips = []
    tril = np.tril(np.ones((B, B), bool))
    for blk in range(NB):
        bs = blk * B
        i, b2 = blk // 2, blk % 2
        for d in range(D):
            apb = ap[bs:bs + B, d]
            Pk = np.ones(B)
            Pk[1:] = np.cumprod(apb[1:])
            with np.errstate(divide="ignore", invalid="ignore"):
                Lb = Pk[:, None] / Pk[None, :]
            Lb = np.nan_to_num(np.where(tril, Lb, 0.0), posinf=0.0, neginf=0.0)
            wdiag[i, :, (b2 * D + d) * B:(b2 * D + d + 1) * B] = \
                Lb.T.astype(np.float16)
            if blk == 0:
                continue
            ps = bs - B
            app = ap[ps:ps + B, d]
            Pp = np.ones(B)
            Pp[1:] = np.cumprod(app[1:])
            with np.errstate(divide="ignore", invalid="ignore"):
                tailp = np.nan_to_num(Pp[B - 1] / Pp, posinf=0.0, neginf=0.0)
            colk = ap[bs, d] * Pk
            Ls = np.outer(tailp, colk)                         # [j, k] lhsT
            longmax = (ap[ps, d] * Pp[B - 1] * colk).max() * Wb[max(ps - 1, 0), d]
            resid_with = (Ls[:B - QR].T @ zb[ps:ps + B - QR, d]).max() + longmax
            resid_no = (Ls.T @ zb[ps:ps + B, d]).max() + longmax
            if resid_no > 2e-4:
                assert resid_with < 2e-3, (
                    f"strip depth {QR} insufficient: {resid_with}")
                strip_mask[blk * D + d] = True
                wstrips.append(Ls[B - QR:B].astype(np.float16))
    nstrip = len(wstrips)
    wstrip = (np.concatenate(wstrips, axis=1) if nstrip
              else np.zeros((QR, B), np.float16))
    return mini, wdiag, np.ascontiguousarray(wstrip), tuple(strip_mask)


def _noise_dev_layout(noise_core):
    """(S_LOCAL, T, D, P) f32 -> (NSUP, 128, 2048) f16, free = (d, s, p)."""
    x = noise_core.transpose(1, 2, 0, 3)           # (T, D, S_LOCAL, P)
    x = x.reshape(NB, B, FB)                       # (blk, tt, dsp)
    x = x.reshape(NSUP, 2, B, FB).transpose(0, 2, 1, 3)  # (i, tt, b2, dsp)
    return np.ascontiguousarray(x.reshape(NSUP, 128, W2_const)).astype(np.float16)


W2_const = 2 * FB


def _undo_layout(dev):
    """(NSUP, 128, 2048) -> (S_LOCAL, T, D, P) float32; free = (d, s, p)."""
    x = dev.reshape(NSUP, B, 2, D, S_LOCAL, P).transpose(0, 2, 1, 3, 4, 5)
    x = x.reshape(T, D, S_LOCAL, P).transpose(2, 0, 1, 3)
    return x.astype(np.float32)


def kernel(
    y=None,
    age=None,
    m=None,
    s_raw=None,
    a_raw=None,
    noise=None,
    cond_sample=None,
    dim_idx=None,
    compute_log_prob=1,
    _trace=False,
    **_unused,
):
    from concourse.bass_utils import run_bass_kernel_spmd

    noise = np.asarray(noise, dtype=np.float32)
    dim_idx = np.asarray(dim_idx)
    mini, wdiag, wstrip, strip_mask = _host_prep(m, s_raw, a_raw, noise, dim_idx)
    nc = _get_nc(strip_mask)

    in_maps = []
    for c in range(N_CORES):
        shard = noise[S_LOCAL * c:S_LOCAL * (c + 1)]
        in_maps.append({
            "noise": _noise_dev_layout(shard),
            "mini": mini,
            "wdiag": wdiag,
            "wstrip": wstrip,
        })

    kw = {}
    if _trace:
        kw = dict(trace=True, trace_cores=list(range(N_CORES)))
    res = run_bass_kernel_spmd(nc, in_maps, core_ids=list(range(N_CORES)), **kw)

    param = np.empty((S, T, D, P), np.float32)
    lp = np.empty((S, T, D, P), np.float32)
    for c in range(N_CORES):
        out = res.results[c]
        sl = slice(S_LOCAL * c, S_LOCAL * (c + 1))
        param[sl] = _undo_layout(out["param"])
        lp[sl] = _undo_layout(out["lp"])
    kernel.last_results = res
    if compute_log_prob:
        return (param, lp)
    return param
